# revision 1
# baseline (speedup 1.0000x reference)
"""Distributed sparse embedding lookup (mean combiner) on 8 Trainium2 cores.

Strategy (data-parallel over output rows, table replicated on every core):
  - Each core owns 1/8 of the output rows (13312 = 104*128). row_indices is
    sorted, so each core's keys are a contiguous slice of the input.
  - Keys are bucketed into 31 vocab windows of 32768 rows (dma_gather index
    tensors are int16). Within a window, keys are split into column-aligned
    chunks such that NO chunk contains two keys of the same output row
    (dma_scatter_add loses updates on duplicate targets within one
    instruction - HW-verified), distributing each row's in-window keys
    round-robin over the window's chunks.
  - Device pipeline per window: dma_gather (random 256B table rows, HBM ->
    SBUF) -> DVE multiply by per-key 1/count (mean pre-scaling, 0-stride
    broadcast along the 64-dim) -> per chunk one dma_scatter_add in
    SBUF-destination parity mode into one of two accumulator pairs
    (alternating, so the WAW serialization chains halve). Accumulator
    layout: output row r -> partition r%128, slot r//128; even slots in
    acc_a*, odd slots in acc_b* of the pair.
  - Final merge: pair0 + pair1 per parity on DVE, then two strided dense
    DMAs into the [13312, 64] output. Host concatenates the 8 core outputs.

All index preprocessing is host-side numpy; all table-data movement and
floating-point arithmetic run on the device.
"""
import numpy as np

_B, _S, _D = 4096, 26, 64
_V = 1_000_000
_M = 8
_R = _B * _S            # 106496 output rows
_RC = _R // _M          # 13312 rows per core = 104 slots * 128
_WIN = 32768
_NWIN = (_V + _WIN - 1) // _WIN      # 31
_ORC = _RC + 128        # +128 pad rows; pads scatter-add into row _RC
_NSLOT = _ORC // 128                 # 105 slots (even: 53, odd: 52)
_BG = 1024              # max num_idxs per dma_gather (HW ring validated)
_BS = 768               # max num_idxs per dma_scatter_add (HW-validated)
_NPAIR = 4              # accumulator pairs (independent WAW chains)

_prog_cache = {}


def _cdiv(a, b):
    return (a + b - 1) // b


def _pack16(v, budget, pad):
    out = np.full(budget, pad, dtype=v.dtype)
    out[: len(v)] = v
    return np.tile(out.reshape(-1, 16).T, (8, 1))


def _pack128(v, budget, pad):
    out = np.full(budget, pad, dtype=v.dtype)
    out[: len(v)] = v
    return out.reshape(-1, 128).T


def _chunk_window(keys, rows, invc, n_chunks, cap):
    """Distribute one window's keys into n_chunks lists, no row repeated
    within a chunk and no chunk above cap. keys are row-major; same-row keys
    are adjacent. Returns None if infeasible with this n_chunks."""
    out_k = [[] for _ in range(n_chunks)]
    out_r = [[] for _ in range(n_chunks)]
    out_i = [[] for _ in range(n_chunks)]
    fill = [0] * n_chunks
    n = len(keys)
    i = 0
    nxt = 0
    while i < n:
        j = i
        r = rows[i]
        while j < n and rows[j] == r:
            j += 1
        used = []
        for t in range(i, j):
            c = None
            for probe in range(n_chunks):
                cand = (nxt + t - i + probe) % n_chunks
                if fill[cand] < cap and cand not in used:
                    c = cand
                    break
            if c is None:
                return None
            used.append(c)
            out_k[c].append(keys[t])
            out_r[c].append(r)
            out_i[c].append(invc[t])
            fill[c] += 1
        nxt = (nxt + 1) % n_chunks
        i = j
    return out_k, out_r, out_i


def _prep(values, row_indices):
    """Returns (gather_budgets, chunk_budgets, in_maps)."""
    values = np.asarray(values).astype(np.int64)
    row_indices = np.asarray(row_indices).astype(np.int64)
    if np.any(np.diff(row_indices) < 0):
        order = np.argsort(row_indices, kind="stable")
        values, row_indices = values[order], row_indices[order]
    bounds = np.searchsorted(row_indices, np.arange(_M + 1) * _RC)
    per_core = []       # per core: per window: (keys, rows, invc)
    for c in range(_M):
        lo, hi = bounds[c], bounds[c + 1]
        keys = values[lo:hi]
        rows = row_indices[lo:hi] - c * _RC
        counts = np.bincount(rows, minlength=_RC).astype(np.float32)
        invc = (1.0 / np.maximum(counts, 1.0))[rows].astype(np.float32)
        # sort by (window, row): row-major within each window
        w = keys // _WIN
        order = np.lexsort((rows, w))
        ks, rs, iv = keys[order], rows[order], invc[order]
        wb = np.searchsorted(ks // _WIN, np.arange(_NWIN + 1))
        wins = []
        for wi in range(_NWIN):
            sl = slice(wb[wi], wb[wi + 1])
            wins.append((ks[sl] - wi * _WIN, rs[sl], iv[sl]))
        per_core.append(wins)

    # per window: number of chunks (same for all cores)
    n_chunks_w = []
    for wi in range(_NWIN):
        need = 1
        for c in range(_M):
            k, r, iv = per_core[c][wi]
            need = max(need, _cdiv(len(k), _BS))
            if len(r):
                _un, cnt = np.unique(r, return_counts=True)
                need = max(need, int(cnt.max()))
        n_chunks_w.append(need)

    # distribute into chunks; chunk budgets = max fill over cores, x128.
    # Raise n_chunks until every core fits the per-instruction cap.
    per_core_chunks = [[None] * _NWIN for _ in range(_M)]
    for wi in range(_NWIN):
        while True:
            ok = True
            for c in range(_M):
                k, r, iv = per_core[c][wi]
                res = _chunk_window(k, r, iv, n_chunks_w[wi], _BS)
                if res is None:
                    ok = False
                    break
                per_core_chunks[c][wi] = res
            if ok:
                break
            n_chunks_w[wi] += 1
    chunk_budgets = []   # flat list over (window, chunk)
    for wi in range(_NWIN):
        for ci in range(n_chunks_w[wi]):
            mx = max(len(per_core_chunks[c][wi][0][ci]) for c in range(_M))
            chunk_budgets.append((wi, max(_cdiv(mx, 128), 1) * 128))

    in_maps = []
    for c in range(_M):
        g_parts, s_parts, i_parts = [], [], []
        ptr = {wi: 0 for wi in range(_NWIN)}
        for wi, bud in chunk_budgets:
            ci = ptr[wi]
            ptr[wi] += 1
            ck, cr, ci_v = per_core_chunks[c][wi]
            k = np.asarray(ck[ci], np.int16)
            r = np.asarray(cr[ci], np.int16)
            iv = np.asarray(ci_v[ci], np.float32)
            g_parts.append(_pack16(k, bud, np.int16(0)))
            s_parts.append(_pack16(r, bud, np.int16(_RC)))  # pad -> dedicated pad slot
            i_parts.append(_pack128(iv, bud, np.float32(0.0)))   # zero contribution
        in_maps.append({
            "gidx": np.ascontiguousarray(np.concatenate(g_parts, axis=1)),
            "sidx": np.ascontiguousarray(np.concatenate(s_parts, axis=1)),
            "invc": np.ascontiguousarray(np.concatenate(i_parts, axis=1)),
        })
    return chunk_budgets, in_maps


def _build(chunk_budgets, n_reps=1):
    from concourse import bacc, mybir, tile

    nc = bacc.Bacc(None, target_bir_lowering=False, debug=False,
                   num_swdge_queues=1)
    table = nc.dram_tensor("table", [_V, _D], mybir.dt.float32,
                           kind="ExternalInput")
    gtot = sum(b // 16 for _w, b in chunk_budgets)
    ntot = sum(b // 128 for _w, b in chunk_budgets)
    gidx = nc.dram_tensor("gidx", [128, gtot], mybir.dt.int16,
                          kind="ExternalInput")
    sidx = nc.dram_tensor("sidx", [128, gtot], mybir.dt.int16,
                          kind="ExternalInput")
    invc = nc.dram_tensor("invc", [128, ntot], mybir.dt.float32,
                          kind="ExternalInput")
    out = nc.dram_tensor("out", [_ORC, _D], mybir.dt.float32,
                         kind="ExternalOutput")
    HGA = (_NSLOT + 1) // 2   # even-slot groups (incl. pad slot)
    HGB = _NSLOT // 2         # odd-slot groups

    with tile.TileContext(nc) as tc:
        with (
            tc.tile_pool(name="acc", bufs=1) as apool,
            tc.tile_pool(name="data", bufs=6) as dpool,
            tc.tile_pool(name="meta", bufs=1) as mpool,
        ):
            accs = []
            for p in range(_NPAIR):
                aa = apool.tile([128, HGA, _D], mybir.dt.float32, tag=f"aa{p}")
                ab = apool.tile([128, HGA, _D], mybir.dt.float32, tag=f"ab{p}")
                nc.vector.memset(aa[:], 0.0)
                nc.vector.memset(ab[:], 0.0)
                accs.append((aa, ab))

            # group consecutive same-window chunks into one gather of <= _BG
            ggroups = []
            for wi, bud in chunk_budgets:
                if (ggroups and ggroups[-1][0] == wi
                        and ggroups[-1][1] + bud <= _BG):
                    ggroups[-1][1] += bud
                    ggroups[-1][2].append(bud)
                else:
                    ggroups.append([wi, bud, [bud]])

            # preload all index/scale metadata once; slice on-chip
            gix = mpool.tile([128, gtot], mybir.dt.int16, tag="gix")
            six = mpool.tile([128, gtot], mybir.dt.int16, tag="six")
            ivx = mpool.tile([128, ntot], mybir.dt.float32, tag="ivx")
            nc.sync.dma_start(out=gix[:], in_=gidx[:])
            nc.sync.dma_start(out=six[:], in_=sidx[:])
            nc.sync.dma_start(out=ivx[:], in_=invc[:])

            for _rep in range(n_reps):
                goff = noff = 0
                chain = 0
                for wi, total, buds in ggroups:
                    nt = total // 128
                    base = wi * _WIN
                    wsize = min(_WIN, _V - base)
                    gat = dpool.tile([128, nt, _D], mybir.dt.float32, tag="gat")
                    nc.gpsimd.dma_gather(
                        out_ap=gat[:], in_ap=table[base:base + wsize, :],
                        idxs_ap=gix[:, goff:goff + total // 16],
                        num_idxs=total, num_idxs_reg=total,
                        elem_size=_D, queue_num=0,
                    )
                    sc = dpool.tile([128, nt, _D], mybir.dt.float32, tag="sc")
                    nc.vector.tensor_tensor(
                        out=sc[:], in0=gat[:],
                        in1=ivx[:, noff:noff + nt, None].to_broadcast(
                            [128, nt, _D]),
                        op=mybir.AluOpType.mult,
                    )
                    coff = 0
                    for bud in buds:
                        aa, ab = accs[chain % _NPAIR]
                        chain += 1
                        nc.gpsimd.dma_scatter_add(
                            out_ap=aa[:], in_ap=sc[:, coff:coff + bud // 128, :],
                            idxs_ap=six[:, goff + coff * 8:
                                        goff + coff * 8 + bud // 16],
                            num_idxs=bud, num_idxs_reg=bud,
                            elem_size=_D, queue_num=0, sbuf_tokens_per_rank=128,
                            parity_reg=0, out_ap_other=ab[:],
                        )
                        coff += bud // 128
                    goff += total // 16
                    noff += nt

            # merge pairs in place into accs[0] and write out
            for par in range(2):
                hg = HGA if par == 0 else HGB
                acc0 = accs[0][par][:, :hg, :]
                for p in range(1, _NPAIR):
                    nc.vector.tensor_add(out=acc0, in0=acc0,
                                         in1=accs[p][par][:, :hg, :])
                out_view = out[:].rearrange("(s p) d -> p s d", p=128)
                nc.sync.dma_start(out=out_view[:, par::2, :], in_=acc0)
    nc.compile()
    return nc


def _state(values, row_indices, emb_table, n_reps=1):
    chunk_budgets, in_maps = _prep(values, row_indices)
    key = (tuple(chunk_budgets), n_reps)
    if key not in _prog_cache:
        _prog_cache[key] = _build(chunk_budgets, n_reps=n_reps)
    nc = _prog_cache[key]
    table = np.ascontiguousarray(np.asarray(emb_table, dtype=np.float32))
    for m in in_maps:
        m["table"] = table
    return nc, in_maps


def kernel(values, row_indices, emb_table):
    from concourse.bass_utils import run_bass_kernel_spmd

    nc, in_maps = _state(values, row_indices, emb_table)
    res = run_bass_kernel_spmd(nc, in_maps, core_ids=list(range(_M)))
    full = np.concatenate(
        [np.asarray(res.results[c]["out"])[:_RC] for c in range(_M)], axis=0)
    return np.ascontiguousarray(full.reshape(_B, _S, _D))

